# revision 48
# baseline (speedup 1.0000x reference)
"""MicroHeadAttention Trainium2 kernel (8-core SPMD, data-parallel over
(batch, row-chunk) pairs).

Shapes (hardcoded): x (2, 2048, 1024), weights (1024, 1024), biases (1024,).
EMBED=1024, 16 heads in 2 blocks (g) of 8 micro-heads, head_dim 64.

Decomposition: the reference's "scramble" is a raw row-major reshape, so the
attention head (b, g, m') consumes exactly rows x[b, 256m':256(m'+1)] and
weight columns [512g:512(g+1)], reshaped (256, 512) -> (2048, 64) with
scrambled position n' = 8*row + m (m = 64-channel sub-block).  16 (b, m')
row-chunks across 8 cores = 2 per core; each chunk has g=0,1 -> 4 heads/core.

Final schedule (measured ~196us max / ~193us mean per-core NEFF vs the
234-249us v2 baseline):
  - hybrid scramble layout: qsc/ksc/vsc columns sorted into position-
    contiguous 128-blocks with m-major order INSIDE each block (col =
    128*(rr//16) + 16*m + rr%16).  Block-level causality means k-blocks
    strictly below the diagonal need no mask, one shared [128,128]
    triangle mask covers every diagonal block (N=128 mask matmuls), and
    the diagonal S/exp/ctx streams skip the above-diagonal quarter of
    each 512-group.  The in-block m-major order keeps the projection
    PSUM->SBUF copies writing 16-element-contiguous runs (a pure
    n'-ordered layout makes them stride-8 scatters: 3.8x slower).
  - ACT (exp) and PE are co-bottlenecks of the attention phase; per-g S
    stages and ctx stages are interleaved [S_g0(t2+1), ctx_g0(t2),
    S_g1(t2+1), ctx_g1(t2)] so the g0 tiles of the next stage (whose
    PSUM banks free when exp(t2, g0) completes mid-stage) are
    compute-ready the moment the ACT queue frees up - ACT never idles.
  - exp stays at [128,1024] per-g granularity (splitting per half costs
    ~155ns/instruction of ACT overhead, +20us measured).
  - all weight DMAs start up front from persistent tiles; no pool-reuse
    dependencies anywhere.  The V projection for the second row-pair
    (p=1) is deferred into the early attention phase (PE slack under
    the ACT-bound cadence), shrinking the serial projection prefix.
  - the PE HAM clock-gate re-throttles to 1.2GHz if PE duty drops during
    the ACT-bound stages and can stay cold for 20-60us; one filler
    matmul per stage plus keep-warm fillers emitted BEFORE (FIFO order!)
    each dependency-blocked tail item keep it at 2.4GHz end-to-end.
  - softmax: denominator rows batched on partitions 0/32/64/96, one
    custom-DVE reciprocal_approx_fast per rc (5x cheaper than
    InstReciprocal), gsel selector matmul broadcasts the rec rows in the
    PE, DVE multiplies them into ctxP.
  - deferred out-proj / rbc emissions are staggered so consecutive users
    of the single psO bank never stall the PE FIFO.
"""

import ml_dtypes
import numpy as np

import concourse.bass as bass
import concourse.mybir as mybir
from concourse import bacc
from concourse.tile import TileContext
from concourse.bass_utils import run_bass_kernel_spmd

F32 = mybir.dt.float32
BF16 = mybir.dt.bfloat16
DT_MM = BF16
NEG = -1e30
E = 1024
R = 512       # rows per core
RP = 256      # rows per pair
ALU = mybir.AluOpType
ACTF = mybir.ActivationFunctionType

_cache = {}


def _build():
    nc = bacc.Bacc()
    xT_d = nc.dram_tensor("xT", (E, R), DT_MM, kind="ExternalInput")
    wq_d = nc.dram_tensor("wqT", (E, E), DT_MM, kind="ExternalInput")
    wk_d = nc.dram_tensor("wkT", (E, E), DT_MM, kind="ExternalInput")
    wv_d = nc.dram_tensor("wvT", (E, E), DT_MM, kind="ExternalInput")
    wo_d = nc.dram_tensor("woTre", (128, 8, E), DT_MM, kind="ExternalInput")
    bq_d = nc.dram_tensor("bqT", (128, 8), F32, kind="ExternalInput")
    bk_d = nc.dram_tensor("bkT8", (128, 8), F32, kind="ExternalInput")
    bv_d = nc.dram_tensor("bvrow", (1, E), F32, kind="ExternalInput")
    bo_d = nc.dram_tensor("borow", (1, E), F32, kind="ExternalInput")
    masks_d = nc.dram_tensor("masks", (128, 128), DT_MM, kind="ExternalInput")
    ident_d = nc.dram_tensor("ident", (128, 128), DT_MM, kind="ExternalInput")
    gsel_d = nc.dram_tensor("gsel", (33, 128), DT_MM, kind="ExternalInput")
    out_d = nc.dram_tensor("out", (R, E), F32, kind="ExternalOutput")

    with TileContext(nc) as tc:
        with (
            tc.tile_pool(name="persist", bufs=1) as pp,
            tc.tile_pool(name="pt", bufs=4) as ptp,
            tc.tile_pool(name="misc", bufs=2) as mp,
            tc.tile_pool(name="outs", bufs=4) as osp,
            tc.tile_pool(name="dram", bufs=1, space="DRAM") as dp,
        ):
            # ---- persistent tiles ----
            bqT = pp.tile([128, 8], F32, tag="bqT", name="bqT")
            bkT8 = pp.tile([128, 8], F32, tag="bkT8", name="bkT8")
            # n'-contiguous layout: qsc/ksc/vsc columns are sorted by the
            # scrambled position n' = 8*rr + m, so causality is
            # block-triangular: k-blocks strictly below the diagonal are
            # fully visible and the single [128,128] upper-triangle mask
            # covers every diagonal block.
            masks = pp.tile([128, 128], DT_MM, tag="masks", name="masks")
            # dependency-free all-zeros warm operand (memset, no DMA)
            wrm = pp.tile([128, 512], DT_MM, tag="wrm", name="wrm")
            ident = pp.tile([128, 128], DT_MM, tag="ident", name="ident")
            gsel = pp.tile([33, 128], DT_MM, tag="gsel", name="gsel")
            # persistent rec-row staging (rows 1-31 stay zero so the 33-wide
            # gsel broadcast matmul never reads uninitialized SBUF)
            reck2s = [pp.tile([33, 512], DT_MM, tag=f"reck2{i}",
                              name=f"reck2{i}") for i in range(2)]
            qsc = pp.tile([128, 4096], DT_MM, tag="qsc", name="qsc")
            ksc = pp.tile([128, 4096], DT_MM, tag="ksc", name="ksc")
            vsc = [[pp.tile([128, 16, 65], DT_MM, tag=f"vsc{p}{g}", name=f"vsc{p}{g}")
                    for g in range(2)] for p in range(2)]
            # ctxP[p][c, rc, m, rr] : out-proj lhsT slices are contiguous
            # (FWL needs a single-stride stationary AP); with m-major ctx
            # columns the divide writes 64-contiguous runs into it
            ctxP = [pp.tile([128, 2, 8, 128], DT_MM, tag=f"ctxP{p}", name=f"ctxP{p}")
                    for p in range(2)]
            vtmp = dp.tile([2, 2, 2048, 64], DT_MM, tag="vtmp", name="vtmp")

            xt = pp.tile([128, 8, R], DT_MM, tag="xt", name="xt")
            wq = pp.tile([128, 8, E], DT_MM, tag="wq", name="wq")
            wk = pp.tile([128, 8, E], DT_MM, tag="wk", name="wk")
            wv = pp.tile([128, 8, E], DT_MM, tag="wv", name="wv")
            wo = pp.tile([128, 8, E], DT_MM, tag="wo", name="wo")
            vnat = [pp.tile([128, 2, E], DT_MM, tag=f"vnat{p}", name=f"vnat{p}")
                    for p in range(2)]
            bvr = pp.tile([1, E], F32, tag="bvr", name="bvr")
            bv_bc = pp.tile([128, E], F32, tag="bvbc", name="bvbc")
            bor = pp.tile([1, E], F32, tag="bor", name="bor")
            bo_bc = pp.tile([128, E], F32, tag="bobc", name="bobc")

            # masks/ident are host constants so the PE pre-warm only
            # waits on these two small DMAs
            nc.sync.dma_start(masks[:], masks_d[:])
            nc.sync.dma_start(ident[:], ident_d[:])
            nc.sync.dma_start(gsel[:], gsel_d[:])
            nc.sync.dma_start(bqT[:], bq_d[:])
            nc.sync.dma_start(bkT8[:], bk_d[:])
            nc.sync.dma_start(bvr[:], bv_d[:])
            nc.sync.dma_start(bor[:], bo_d[:])

            nc.sync.dma_start(xt[:], xT_d.rearrange("(ko ki) r -> ki ko r", ki=128))
            wv_v = wv_d.rearrange("(ko ki) o -> ki ko o", ki=128)
            # oc-major so V's first output half can start after 1 MB
            for oc in range(2):
                nc.sync.dma_start(wv[:, :, 512 * oc:512 * (oc + 1)],
                                  wv_v[:, :, 512 * oc:512 * (oc + 1)])
            nc.sync.dma_start(wq[:], wq_d.rearrange("(ko ki) o -> ki ko o", ki=128))
            nc.sync.dma_start(wk[:], wk_d.rearrange("(ko ki) o -> ki ko o", ki=128))
            nc.sync.dma_start(wo[:], wo_d[:])

            nc.gpsimd.partition_broadcast(bv_bc[:], bvr[:])
            nc.gpsimd.partition_broadcast(bo_bc[:], bor[:])
            ones16 = pp.tile([128, 16], F32, tag="ones16", name="ones16")
            nc.gpsimd.memset(ones16[:], 1.0)
            nc.vector.memset(wrm[:], 0.0)
            for i in range(2):
                nc.vector.memset(reck2s[i][:], 0.0)
            for p in range(2):
                for g in range(2):
                    nc.vector.tensor_copy(vsc[p][g][:, :, 64], ones16[:])

            def v_group(oc, rc, psp_, tag="psA"):
                p, half = rc // 2, rc % 2
                ps = psp_.tile([128, 512], F32, tag=tag, name="psA")
                for ki in range(8):
                    nc.tensor.matmul(
                        ps[:], xt[:, ki, 128 * rc:128 * (rc + 1)],
                        wv[:, ki, 512 * oc:512 * (oc + 1)],
                        start=(ki == 0), stop=(ki == 7))
                nc.vector.tensor_tensor(
                    vnat[p][:, half, 512 * oc:512 * (oc + 1)],
                    ps[:], bv_bc[:, 512 * oc:512 * (oc + 1)], ALU.add)

            def v_scramble(p):
                for g in range(2):
                    # n' = 8*(128h + 64rb + rr) + m = 1024h + 512rb
                    # + 8rr + m.  Split per (h, rb): the DMA balancer
                    # tops out at 3 dims.
                    dstv = vtmp[p, g].rearrange(
                        "(h rb rr m) d -> h rb rr m d", h=2, rb=2, m=8)
                    for h in range(2):
                        for rb in range(2):
                            srcs = vnat[p][64 * rb:64 * (rb + 1), h,
                                           512 * g:512 * (g + 1)]
                            nc.sync.dma_start(
                                dstv[h, rb],
                                srcs.rearrange("rr (m d) -> rr m d", m=8))
                for g in range(2):
                    # vsc partition index is the in-block m-major coord
                    # 16m + rr%16; vtmp rows are flat n' = 128 kb + 8 rr
                    # + m, so gather per m to keep the AP affine
                    src_v = vtmp[p, g].rearrange(
                        "(kb rr mm) d -> rr mm kb d", kb=16, rr=16)
                    for m in range(8):
                        nc.sync.dma_start(
                            vsc[p][g][16 * m:16 * (m + 1), :, 0:64],
                            src_v[:, m])

            with tc.tile_pool(name="ps1", bufs=5, space="PSUM") as psp, \
                 tc.tile_pool(name="pswarm", bufs=1, space="PSUM") as pwp:
                # ---- PE pre-warm: back-to-back matmuls on the mask tile
                # keep the HAM activity window busy while the input DMAs
                # stream, so the real matmuls start at 2.4 GHz.
                psw = pwp.tile([128, 512], F32, tag="psw", name="psw")
                for _ in range(24):
                    nc.tensor.matmul(psw[:], ident[:], wrm[:],
                                     start=True, stop=True)

                def qk_proj(w_tile, bias_tile, scale, dst):
                    for t in range(8):
                        ps = psp.tile([128, 512], F32, tag="psA", name="psA")
                        for ki in range(8):
                            nc.tensor.matmul(
                                ps[:], w_tile[:, ki, 128 * t:128 * (t + 1)],
                                xt[:, ki, :], start=(ki == 0), stop=(ki == 7))
                        g, u = t // 4, t % 4
                        for mh in range(2):
                            mmv = 2 * u + mh
                            # position-sorted 128-blocks (b = rr//16),
                            # m-major inside the block: col = 2048 pp
                            # + 128 b + 16 m + rr%16 - head m's channels
                            # land in contiguous 16-element runs
                            dest = dst.rearrange(
                                "c (pp b mm rrlo) -> c pp b mm rrlo",
                                pp=2, b=16, mm=8, rrlo=16)[
                                64 * g:64 * (g + 1), :, :, mmv, :]
                            src = ps[64 * mh:64 * (mh + 1), :].rearrange(
                                "c (pp b rrlo) -> c pp b rrlo",
                                pp=2, b=16)
                            # VectorE lanes are partition-locked: it can only
                            # take the copies whose src/dst partition ranges
                            # line up (g == mh); ScalarE handles the crossed
                            # ones.
                            if mh != g:
                                nc.scalar.activation(
                                    dest, src, ACTF.Identity,
                                    bias=bias_tile[64 * mh:64 * (mh + 1), t:t + 1],
                                    scale=scale)
                            else:
                                nc.vector.tensor_scalar(
                                    out=dest, in0=src, scalar1=scale,
                                    scalar2=bias_tile[64 * mh:64 * (mh + 1), t:t + 1],
                                    op0=ALU.mult, op1=ALU.add)

                # V projection for the first row-pair only; p=1 is deferred
                # into the attention phase (PE slack under the ACT-bound
                # cadence).
                for oc in range(2):
                    for rc in range(2):
                        v_group(oc, rc, psp)
                v_scramble(0)

                qk_proj(wq, bqT, 1.0, qsc)
                qk_proj(wk, bkT8, 0.125, ksc)

            # ---- attention + interleaved output projection ----
            with tc.tile_pool(name="psS", bufs=2, space="PSUM") as pssp, \
                 tc.tile_pool(name="psctx", bufs=2, space="PSUM") as pcp, \
                 tc.tile_pool(name="psO", bufs=1, space="PSUM") as psop, \
                 tc.tile_pool(name="pswarm2", bufs=1, space="PSUM") as pwp2:

                warm_ps = pwp2.tile([128, 512], F32, tag="warm", name="warm")

                def warm(n):
                    for _ in range(n):
                        nc.tensor.matmul(warm_ps[:], ident[:], wrm[:],
                                         start=True, stop=True)

                def out_proj(p, rc, oc):
                    ps = psop.tile([128, 512], F32, tag="psO", name="psO")
                    for mmv in range(8):
                        nc.tensor.matmul(
                            ps[:],
                            ctxP[p][:, rc, mmv, :],
                            wo[:, mmv, 512 * oc:512 * (oc + 1)],
                            start=(mmv == 0), stop=(mmv == 7))
                    outsb = osp.tile([128, 512], F32, tag="outsb",
                                     name="outsb")
                    nc.vector.tensor_tensor(
                        outsb[:], ps[:],
                        bo_bc[:, 512 * oc:512 * (oc + 1)], ALU.add)
                    nc.sync.dma_start(
                        out_d[RP * p + 128 * rc:RP * p + 128 * (rc + 1),
                              512 * oc:512 * (oc + 1)],
                        outsb[:])

                # deferred emissions: [countdown_in_t2_steps, fn]
                pending = []

                def drain():
                    for item in pending[:]:
                        item[0] -= 1
                        if item[0] <= 0:
                            pending.remove(item)
                            item[1]()

                # V projection p=1 + scramble, spread over the early
                # attention stages (psO bank is idle there; out-proj
                # deferrals only begin after the first rc completes)
                for i, (oc, rc) in enumerate([(0, 2), (0, 3), (1, 2), (1, 3)]):
                    pending.append(
                        [1 + 3 * i,
                         lambda oc=oc, rc=rc: v_group(oc, rc, psop, "psO")])
                pending.append([13, lambda: v_scramble(1)])

                def mk_divide(p, rc, box, tail=False):
                    def divide(jh2):
                        recS = box[0]
                        # rec rows for both g at partitions 0/32,
                        # then ONE PE matmul with the gsel selector
                        # broadcasts them to partitions 0-63 /
                        # 64-127 in PSUM.  In the latency-bound final
                        # chain the copies go to the (idle) ACT engine
                        # so they run in parallel with the DVE work.
                        reck2 = reck2s[jh2]
                        for g in range(2):
                            k4 = 2 * jh2 + g
                            if tail:
                                nc.scalar.activation(
                                    reck2[32 * g:32 * g + 1, :],
                                    recS[32 * k4:32 * k4 + 1, :],
                                    ACTF.Identity)
                            else:
                                nc.vector.tensor_copy(
                                    reck2[32 * g:32 * g + 1, :],
                                    recS[32 * k4:32 * k4 + 1, :])
                        rbc_ps = psop.tile([128, 512], F32,
                                           tag="psO", name="rbcps")
                        nc.tensor.matmul(
                            rbc_ps[:], gsel[:], reck2[0:33, :],
                            start=True, stop=True)
                        if tail:
                            # keep-warm matmuls with a REAL dependency on
                            # reck2: they execute here in the chain, not
                            # up-front like dependency-free fillers, so
                            # the PE clock stays at 2.4GHz through the
                            # DVE multiplies and into the out-projection
                            for _ in range(4):
                                nc.tensor.matmul(
                                    warm_ps[:], gsel[:], reck2[0:33, :],
                                    start=True, stop=True)
                        for g in range(2):
                            dst = ctxP[p][64 * g:64 * (g + 1), rc, :,
                                          64 * jh2:64 * (jh2 + 1)]
                            nc.vector.tensor_tensor(
                                dst.rearrange(
                                    "c m (b rr) -> c m b rr", b=4),
                                dst.rearrange(
                                    "c m (b rr) -> c m b rr", b=4),
                                rbc_ps[64 * g:64 * (g + 1), :]
                                .rearrange(
                                    "c (b m rr) -> c m b rr",
                                    b=4, m=8),
                                ALU.mult)
                    return divide

                pending_tail = []
                for p in range(2):
                    denS = None
                    # descending j5: pairs (3,2) then (1,0), so each
                    # pair-completion divide is covered by a LONG next
                    # group (the p-transition lands on nt2=8, not 2)
                    for j5 in (3, 2, 1, 0):
                        nt2 = 2 * (j5 + 1)   # pairs of 128-wide k blocks
                        jh = j5 % 2
                        if jh == 1:
                            # 4 denominator rows (jh, g) staged on
                            # separate partitions; the jh=1 half's
                            # reciprocal + divide fire as soon as THIS
                            # group's tail lands (during the next group),
                            # pulling two hops out of the final chain
                            denS = mp.tile([128, 512], F32, tag="denS",
                                           name="denS")
                        ctx_ps = [pcp.tile([65, 512], F32, tag="ctxps",
                                           name="ctxps")
                                  for _ in range(2)]
                        pts = [None] * nt2

                        def s_stage_g(t2, g):
                            # S for one g: two 512-col halves into one
                            # [128,1024] PSUM tile; diagonal k-blocks only
                            # stream the causal q range (cols >= the
                            # block's position offset) and get the shared
                            # [128,128] triangle mask via an N=128
                            # identity matmul, emitted after both S halves
                            # so the same-bank accumulation never waits on
                            # an undrained S write.
                            st = pssp.tile([128, 1024], F32, tag="st",
                                           name="st")
                            pt = ptp.tile([128, 1024], DT_MM, tag="pt",
                                          name="pt")
                            diag = t2 >= 2 * j5
                            for half in range(2):
                                kb = 2 * t2 + half
                                off = 128 * (kb - 4 * j5) if diag else 0
                                nc.tensor.matmul(
                                    st[:, 512 * half + off:
                                       512 * (half + 1)],
                                    ksc[64 * g:64 * (g + 1),
                                        2048 * p + 128 * kb:
                                        2048 * p + 128 * (kb + 1)],
                                    qsc[64 * g:64 * (g + 1),
                                        2048 * p + 512 * j5 + off:
                                        2048 * p + 512 * (j5 + 1)],
                                    start=True, stop=not diag)
                            if diag:
                                for half in range(2):
                                    kb = 2 * t2 + half
                                    off = 128 * (kb - 4 * j5)
                                    nc.tensor.matmul(
                                        st[:, 512 * half + off:
                                           512 * half + off + 128],
                                        ident[:], masks[:],
                                        start=False, stop=True)
                                for half in range(2):
                                    off = 128 * (2 * t2 + half - 4 * j5)
                                    nc.scalar.activation(
                                        pt[:, 512 * half + off:
                                           512 * (half + 1)],
                                        st[:, 512 * half + off:
                                           512 * (half + 1)], ACTF.Exp)
                            else:
                                nc.scalar.activation(pt[:], st[:], ACTF.Exp)
                            return pt

                        def ctx_stage_g(t2, g, pts=pts, ctx_ps=ctx_ps, p=p,
                                        nt2=nt2, j5=j5):
                            for half in range(2):
                                kb = 2 * t2 + half
                                off = (128 * (kb - 4 * j5)
                                       if kb >= 4 * j5 else 0)
                                nc.tensor.matmul(
                                    ctx_ps[g][:, off:512],
                                    vsc[p][g][:, kb, :],
                                    pts[t2][g][:, 512 * half + off:
                                               512 * (half + 1)],
                                    start=(kb == 0),
                                    stop=(kb == 2 * nt2 - 1))

                        # one-stage software pipeline, interleaved per g:
                        # [S_g0(t2), ctx_g0(t2-1), S_g1(t2), ctx_g1(t2-1)].
                        # S_g(t2)'s PSUM bank frees when exp(t2-1, g)
                        # completes, so the g0 work is compute-ready
                        # mid-stage and exp(t2, g0) starts the moment the
                        # ACT queue frees - ACT never idles.
                        for t2 in range(nt2):
                            pts[t2] = [None, None]
                            for g in range(2):
                                pts[t2][g] = s_stage_g(t2, g)
                                if t2 == 0 and g == 0 and pending_tail:
                                    # previous group's last ctx + evacuation
                                    # runs under this group's first S tiles
                                    pending_tail.pop()()
                                if t2 >= 1:
                                    ctx_stage_g(t2 - 1, g)
                            # one filler matmul per stage: the ACT-bound
                            # cadence leaves the PE under the HAM activity
                            # threshold on the lighter (diagonal/short)
                            # stages, and a single re-throttle costs far
                            # more than 215ns/stage of filler
                            warm(1)
                            drain()

                        final_grp = (p == 1 and j5 == 0)

                        def group_tail(j5=j5, jh=jh, ctx_ps=ctx_ps,
                                       nt2=nt2, denS=denS, p=p,
                                       ctx_stage_g=ctx_stage_g,
                                       final=final_grp):
                            for g in range(2):
                                ctx_stage_g(nt2 - 1, g)
                            # evacuate PSUM fast (frees the ctx banks for
                            # the next group); the reciprocal/divide runs
                            # later, overlapped under later compute.  For
                            # the final group the den copies go to the
                            # idle ACT engine, halving the serial DVE
                            # latency of the closing chain.
                            for g in range(2):
                                # [c, rc, m, 64jh + 16b + rrlo] <- ctx col
                                # (128b + 16m + rrlo)
                                nc.vector.tensor_copy(
                                    ctxP[p][64 * g:64 * (g + 1), j5 // 2, :,
                                            64 * jh:64 * (jh + 1)]
                                    .rearrange("c m (b rr) -> c m b rr",
                                               b=4),
                                    ctx_ps[g][0:64, :].rearrange(
                                        "c (b m rr) -> c m b rr",
                                        b=4, m=8))
                                if final:
                                    nc.scalar.activation(
                                        denS[32 * (2 * jh + g):
                                             32 * (2 * jh + g) + 1, :],
                                        ctx_ps[g][64:65, :], ACTF.Identity)
                                else:
                                    nc.vector.tensor_copy(
                                        denS[32 * (2 * jh + g):
                                             32 * (2 * jh + g) + 1, :],
                                        ctx_ps[g][64:65, :])
                            if final:
                                # warm anchors on the just-evacuated ctxP:
                                # run mid-chain, bridging the reciprocal
                                for _ in range(4):
                                    nc.tensor.matmul(
                                        warm_ps[:],
                                        ctxP[p][:, j5 // 2, 0, :],
                                        wo[:, 0, 0:512],
                                        start=True, stop=True)

                        pending_tail.append(group_tail)

                        if jh == 0:
                            rc = j5 // 2
                            box = []

                            def mkrec(denS=denS, box=box):
                                recS = mp.tile([128, 512], F32, tag="recS",
                                               name="recS")
                                nc.vector.reciprocal_approx_fast(
                                    recS[:], denS[:])
                                box.append(recS)

                            div = mk_divide(p, rc, box,
                                            tail=(p == 1 and rc == 0))
                            if p == 0:
                                cds = (3, 3, 5, 7, 9)
                            else:
                                cds = (2, 2, 3, 4, 6)
                            pending.append([cds[0], mkrec])
                            pending.append([cds[1], lambda d=div: d(0)])
                            pending.append([cds[2], lambda d=div: d(1)])
                            pending.append(
                                [cds[3],
                                 lambda p=p, rc=rc: out_proj(p, rc, 0)])
                            pending.append(
                                [cds[4],
                                 lambda p=p, rc=rc: out_proj(p, rc, 1)])
                while pending_tail:
                    warm(4)
                    pending_tail.pop()()
                for item in pending:
                    warm(6)
                    item[1]()
                warm(4)

    nc.compile()
    return nc


def _get_nc():
    key = "nc"
    if key not in _cache:
        _cache[key] = _build()
    return _cache[key]


def pack_in_maps(x, Wq, bq, Wk, bk, Wv, bv, Wo, bo):
    BF = ml_dtypes.bfloat16
    x = np.asarray(x, np.float32)
    WqT = np.ascontiguousarray(np.asarray(Wq, np.float32).T.astype(BF))
    WkT = np.ascontiguousarray(np.asarray(Wk, np.float32).T.astype(BF))
    WvT = np.ascontiguousarray(np.asarray(Wv, np.float32).T.astype(BF))
    # woTre[64g + d, m, o] = Wo[o, 512g + 64m + d]
    WoTre = np.ascontiguousarray(
        np.asarray(Wo, np.float32).T.reshape(2, 8, 64, E).transpose(0, 2, 1, 3)
        .reshape(128, 8, E).astype(BF))
    bqT = np.ascontiguousarray(np.asarray(bq, np.float32).reshape(8, 128).T)
    bkT8 = np.ascontiguousarray((np.asarray(bk, np.float32) / 8.0).reshape(8, 128).T)
    bvrow = np.asarray(bv, np.float32).reshape(1, E)
    borow = np.asarray(bo, np.float32).reshape(1, E)
    # position-sorted 128-blocks, m-major in-block: index i = 16m + rr%16
    # has in-block position 8*(i%16) + i//16; one mask covers every
    # diagonal block
    ii = np.arange(128)[:, None]
    cc = np.arange(128)[None, :]
    pos_k = 8 * (ii % 16) + ii // 16
    pos_q = 8 * (cc % 16) + cc // 16
    masks = np.where(pos_k <= pos_q, 0.0, NEG).astype(BF)
    ident = np.eye(128).astype(BF)
    gsel = np.zeros((33, 128), np.float32)
    gsel[0, 0:64] = 1.0
    gsel[32, 64:128] = 1.0
    gsel = gsel.astype(BF)

    in_maps = []
    for c in range(8):
        xTs = np.empty((E, R), BF)
        for p in range(2):
            h = 2 * c + p
            b_, mp_ = divmod(h, 8)
            xTs[:, RP * p:RP * (p + 1)] = x[b_, RP * mp_:RP * (mp_ + 1), :].T.astype(BF)
        in_maps.append({
            "xT": np.ascontiguousarray(xTs), "wqT": WqT, "wkT": WkT,
            "wvT": WvT, "woTre": WoTre, "bqT": bqT, "bkT8": bkT8,
            "bvrow": bvrow, "borow": borow, "masks": masks, "ident": ident,
            "gsel": gsel,
        })
    return in_maps


def unpack_out(results):
    out = np.empty((2, 2048, E), np.float32)
    for c in range(8):
        o = results[c]["out"]
        for p in range(2):
            h = 2 * c + p
            b_, mp_ = divmod(h, 8)
            out[b_, RP * mp_:RP * (mp_ + 1), :] = o[RP * p:RP * (p + 1), :]
    return out


def kernel(x, Wq, bq, Wk, bk, Wv, bv, Wo, bo):
    in_maps = pack_in_maps(x, Wq, bq, Wk, bk, Wv, bv, Wo, bo)
    nc = _get_nc()
    res = run_bass_kernel_spmd(nc, in_maps, core_ids=list(range(8)))
    return unpack_out(res.results)


# revision 53
# speedup vs baseline: 1.2671x; 1.2671x over previous
"""MicroHeadAttention Trainium2 kernel (8-core SPMD, data-parallel over
(batch, row-chunk) pairs).

Shapes (hardcoded): x (2, 2048, 1024), weights (1024, 1024), biases (1024,).
EMBED=1024, 16 heads in 2 blocks (g) of 8 micro-heads, head_dim 64.

Decomposition: the reference's "scramble" is a raw row-major reshape, so the
attention head (b, g, m') consumes exactly rows x[b, 256m':256(m'+1)] and
weight columns [512g:512(g+1)], reshaped (256, 512) -> (2048, 64) with
scrambled position n' = 8*row + m (m = 64-channel sub-block).  16 (b, m')
row-chunks across 8 cores = 2 per core; each chunk has g=0,1 -> 4 heads/core.

Final schedule (~196us max / ~193us mean per-core NEFF, vs 234-249us
baseline).  Layout: position-sorted 128-blocks with m-major order INSIDE
each block (col = 128*(rr//16) + 16*m + rr%16) - block-triangular
causality (diagonal S/exp/ctx skip the above-diagonal quarter, one shared
[128,128] triangle mask) while projection PSUM evacuations still write
16-element-contiguous runs (pure n'-order makes them 3.8x-slower stride-8
scatters).  One filler matmul per attention stage keeps the PE above the
HAM clock-gate activity threshold (a single re-throttle to 1.2GHz costs
20-60us).  Schedule notes:
  - ACT (exp) is the bottleneck engine of the attention phase (~91us of
    exp at ~1.1us per [128,1024] tile); everything is arranged so ACT never
    waits: per-g S stages and ctx stages are interleaved
    [S_g0(t2+1), ctx_g0(t2), S_g1(t2+1), ctx_g1(t2)] so the g0 tiles of the
    next stage (whose PSUM banks free when exp(t2, g0) completes mid-stage)
    are compute-ready the moment the ACT queue frees up.
  - exp stays at [128,1024] per-g granularity: splitting it per half costs
    ~155ns/instruction of ACT overhead (+20us measured in v3).
  - all weight DMAs start up front from persistent tiles (single 2MB
    transfers); no pool-reuse dependencies anywhere.
  - the V projection for the second row-pair (p=1) is deferred into the
    early attention phase (PE slack under the ACT-bound cadence), shrinking
    the serial projection prefix.
  - softmax divide: denominator rows batched on partitions 0/32/64/96,
    one custom-DVE reciprocal_approx_fast per rc (5x cheaper than
    InstReciprocal), a gsel selector matmul broadcasts the rec rows on
    the PE, DVE multiplies them into ctxP.
  - deferred out-proj / rbc emissions are staggered so consecutive users of
    the single psO bank never stall the PE FIFO; the final drain interleaves
    keep-warm matmuls on a spare PSUM bank so the latency-bound tail chain
    runs at 2.4GHz.
"""

import ml_dtypes
import numpy as np

import concourse.bass as bass
import concourse.mybir as mybir
from concourse import bacc
from concourse.tile import TileContext
from concourse.bass_utils import run_bass_kernel_spmd

F32 = mybir.dt.float32
BF16 = mybir.dt.bfloat16
DT_MM = BF16
NEG = -1e30
E = 1024
R = 512       # rows per core
RP = 256      # rows per pair
ALU = mybir.AluOpType
ACTF = mybir.ActivationFunctionType

_cache = {}


def _build():
    nc = bacc.Bacc()
    xT_d = nc.dram_tensor("xT", (E, R), DT_MM, kind="ExternalInput")
    wq_d = nc.dram_tensor("wqT", (E, E), DT_MM, kind="ExternalInput")
    wk_d = nc.dram_tensor("wkT", (E, E), DT_MM, kind="ExternalInput")
    wv_d = nc.dram_tensor("wvT", (E, E), DT_MM, kind="ExternalInput")
    wo_d = nc.dram_tensor("woTre", (128, 8, E), DT_MM, kind="ExternalInput")
    bq_d = nc.dram_tensor("bqT", (128, 8), F32, kind="ExternalInput")
    bk_d = nc.dram_tensor("bkT8", (128, 8), F32, kind="ExternalInput")
    bv_d = nc.dram_tensor("bvrow", (1, E), F32, kind="ExternalInput")
    bo_d = nc.dram_tensor("borow", (1, E), F32, kind="ExternalInput")
    masks_d = nc.dram_tensor("masks", (128, 128), DT_MM, kind="ExternalInput")
    ident_d = nc.dram_tensor("ident", (128, 128), DT_MM, kind="ExternalInput")
    gsel_d = nc.dram_tensor("gsel", (33, 128), DT_MM, kind="ExternalInput")
    out_d = nc.dram_tensor("out", (R, E), F32, kind="ExternalOutput")

    with TileContext(nc) as tc:
        with (
            tc.tile_pool(name="persist", bufs=1) as pp,
            tc.tile_pool(name="pt", bufs=4) as ptp,
            tc.tile_pool(name="misc", bufs=2) as mp,
            tc.tile_pool(name="outs", bufs=4) as osp,
            tc.tile_pool(name="dram", bufs=1, space="DRAM") as dp,
        ):
            # ---- persistent tiles ----
            bqT = pp.tile([128, 8], F32, tag="bqT", name="bqT")
            bkT8 = pp.tile([128, 8], F32, tag="bkT8", name="bkT8")
            # n'-contiguous layout: qsc/ksc/vsc columns are sorted by the
            # scrambled position n' = 8*rr + m, so causality is
            # block-triangular: k-blocks strictly below the diagonal are
            # fully visible and the single [128,128] upper-triangle mask
            # covers every diagonal block.
            masks = pp.tile([128, 128], DT_MM, tag="masks", name="masks")
            # dependency-free all-zeros warm operand (memset, no DMA)
            wrm = pp.tile([128, 512], DT_MM, tag="wrm", name="wrm")
            ident = pp.tile([128, 128], DT_MM, tag="ident", name="ident")
            gsel = pp.tile([33, 128], DT_MM, tag="gsel", name="gsel")
            # persistent rec-row staging (rows 1-31 stay zero so the 33-wide
            # gsel broadcast matmul never reads uninitialized SBUF)
            reck2s = [pp.tile([33, 512], DT_MM, tag=f"reck2{i}",
                              name=f"reck2{i}") for i in range(2)]
            qsc = pp.tile([128, 4096], DT_MM, tag="qsc", name="qsc")
            ksc = pp.tile([128, 4096], DT_MM, tag="ksc", name="ksc")
            vsc = [[pp.tile([128, 16, 65], DT_MM, tag=f"vsc{p}{g}", name=f"vsc{p}{g}")
                    for g in range(2)] for p in range(2)]
            # ctxP[p][c, rc, m, rr] : out-proj lhsT slices are contiguous
            # (FWL needs a single-stride stationary AP); with m-major ctx
            # columns the divide writes 64-contiguous runs into it
            ctxP = [pp.tile([128, 2, 8, 128], DT_MM, tag=f"ctxP{p}", name=f"ctxP{p}")
                    for p in range(2)]
            vtmp = dp.tile([2, 2, 2048, 64], DT_MM, tag="vtmp", name="vtmp")

            xt = pp.tile([128, 8, R], DT_MM, tag="xt", name="xt")
            wq = pp.tile([128, 8, E], DT_MM, tag="wq", name="wq")
            wk = pp.tile([128, 8, E], DT_MM, tag="wk", name="wk")
            wv = pp.tile([128, 8, E], DT_MM, tag="wv", name="wv")
            wo = pp.tile([128, 8, E], DT_MM, tag="wo", name="wo")
            vnat = [pp.tile([128, 2, E], DT_MM, tag=f"vnat{p}", name=f"vnat{p}")
                    for p in range(2)]
            bvr = pp.tile([1, E], F32, tag="bvr", name="bvr")
            bv_bc = pp.tile([128, E], F32, tag="bvbc", name="bvbc")
            bor = pp.tile([1, E], F32, tag="bor", name="bor")
            bo_bc = pp.tile([128, E], F32, tag="bobc", name="bobc")

            # masks/ident are host constants so the PE pre-warm only
            # waits on these two small DMAs
            nc.sync.dma_start(masks[:], masks_d[:])
            nc.sync.dma_start(ident[:], ident_d[:])
            nc.sync.dma_start(gsel[:], gsel_d[:])
            nc.sync.dma_start(bqT[:], bq_d[:])
            nc.sync.dma_start(bkT8[:], bk_d[:])
            nc.sync.dma_start(bvr[:], bv_d[:])
            nc.sync.dma_start(bor[:], bo_d[:])

            nc.sync.dma_start(xt[:], xT_d.rearrange("(ko ki) r -> ki ko r", ki=128))
            wv_v = wv_d.rearrange("(ko ki) o -> ki ko o", ki=128)
            # oc-major so V's first output half can start after 1 MB
            for oc in range(2):
                nc.sync.dma_start(wv[:, :, 512 * oc:512 * (oc + 1)],
                                  wv_v[:, :, 512 * oc:512 * (oc + 1)])
            nc.sync.dma_start(wq[:], wq_d.rearrange("(ko ki) o -> ki ko o", ki=128))
            nc.sync.dma_start(wk[:], wk_d.rearrange("(ko ki) o -> ki ko o", ki=128))
            nc.sync.dma_start(wo[:], wo_d[:])

            nc.gpsimd.partition_broadcast(bv_bc[:], bvr[:])
            nc.gpsimd.partition_broadcast(bo_bc[:], bor[:])
            ones16 = pp.tile([128, 16], F32, tag="ones16", name="ones16")
            nc.gpsimd.memset(ones16[:], 1.0)
            nc.vector.memset(wrm[:], 0.0)
            for i in range(2):
                nc.vector.memset(reck2s[i][:], 0.0)
            for p in range(2):
                for g in range(2):
                    nc.vector.tensor_copy(vsc[p][g][:, :, 64], ones16[:])

            def v_group(oc, rc, psp_, tag="psA"):
                p, half = rc // 2, rc % 2
                ps = psp_.tile([128, 512], F32, tag=tag, name="psA")
                for ki in range(8):
                    nc.tensor.matmul(
                        ps[:], xt[:, ki, 128 * rc:128 * (rc + 1)],
                        wv[:, ki, 512 * oc:512 * (oc + 1)],
                        start=(ki == 0), stop=(ki == 7))
                nc.vector.tensor_tensor(
                    vnat[p][:, half, 512 * oc:512 * (oc + 1)],
                    ps[:], bv_bc[:, 512 * oc:512 * (oc + 1)], ALU.add)

            def v_scramble(p):
                for g in range(2):
                    # n' = 8*(128h + 64rb + rr) + m = 1024h + 512rb
                    # + 8rr + m.  Split per (h, rb): the DMA balancer
                    # tops out at 3 dims.
                    dstv = vtmp[p, g].rearrange(
                        "(h rb rr m) d -> h rb rr m d", h=2, rb=2, m=8)
                    for h in range(2):
                        for rb in range(2):
                            srcs = vnat[p][64 * rb:64 * (rb + 1), h,
                                           512 * g:512 * (g + 1)]
                            nc.sync.dma_start(
                                dstv[h, rb],
                                srcs.rearrange("rr (m d) -> rr m d", m=8))
                for g in range(2):
                    # vsc partition index is the in-block m-major coord
                    # 16m + rr%16; vtmp rows are flat n' = 128 kb + 8 rr
                    # + m, so gather per m to keep the AP affine
                    src_v = vtmp[p, g].rearrange(
                        "(kb rr mm) d -> rr mm kb d", kb=16, rr=16)
                    for m in range(8):
                        nc.sync.dma_start(
                            vsc[p][g][16 * m:16 * (m + 1), :, 0:64],
                            src_v[:, m])

            with tc.tile_pool(name="ps1", bufs=5, space="PSUM") as psp, \
                 tc.tile_pool(name="pswarm", bufs=1, space="PSUM") as pwp:
                # ---- PE pre-warm: back-to-back matmuls on the mask tile
                # keep the HAM activity window busy while the input DMAs
                # stream, so the real matmuls start at 2.4 GHz.
                psw = pwp.tile([128, 512], F32, tag="psw", name="psw")
                for _ in range(21):
                    nc.tensor.matmul(psw[:], ident[:], wrm[:],
                                     start=True, stop=True)

                def qk_proj(w_tile, bias_tile, scale, dst):
                    for t in range(8):
                        ps = psp.tile([128, 512], F32, tag="psA", name="psA")
                        for ki in range(8):
                            nc.tensor.matmul(
                                ps[:], w_tile[:, ki, 128 * t:128 * (t + 1)],
                                xt[:, ki, :], start=(ki == 0), stop=(ki == 7))
                        g, u = t // 4, t % 4
                        for mh in range(2):
                            mmv = 2 * u + mh
                            # position-sorted 128-blocks (b = rr//16),
                            # m-major inside the block: col = 2048 pp
                            # + 128 b + 16 m + rr%16 - head m's channels
                            # land in contiguous 16-element runs
                            dest = dst.rearrange(
                                "c (pp b mm rrlo) -> c pp b mm rrlo",
                                pp=2, b=16, mm=8, rrlo=16)[
                                64 * g:64 * (g + 1), :, :, mmv, :]
                            src = ps[64 * mh:64 * (mh + 1), :].rearrange(
                                "c (pp b rrlo) -> c pp b rrlo",
                                pp=2, b=16)
                            # VectorE lanes are partition-locked: it can only
                            # take the copies whose src/dst partition ranges
                            # line up (g == mh); ScalarE handles the crossed
                            # ones.
                            if mh != g:
                                nc.scalar.activation(
                                    dest, src, ACTF.Identity,
                                    bias=bias_tile[64 * mh:64 * (mh + 1), t:t + 1],
                                    scale=scale)
                            else:
                                nc.vector.tensor_scalar(
                                    out=dest, in0=src, scalar1=scale,
                                    scalar2=bias_tile[64 * mh:64 * (mh + 1), t:t + 1],
                                    op0=ALU.mult, op1=ALU.add)

                # V projection for the first row-pair only; p=1 is deferred
                # into the attention phase (PE slack under the ACT-bound
                # cadence).
                for oc in range(2):
                    for rc in range(2):
                        v_group(oc, rc, psp)
                v_scramble(0)

                qk_proj(wq, bqT, 1.0, qsc)
                qk_proj(wk, bkT8, 0.125, ksc)

            # ---- attention + interleaved output projection ----
            with tc.tile_pool(name="psS", bufs=2, space="PSUM") as pssp, \
                 tc.tile_pool(name="psctx", bufs=2, space="PSUM") as pcp, \
                 tc.tile_pool(name="psO", bufs=1, space="PSUM") as psop, \
                 tc.tile_pool(name="pswarm2", bufs=1, space="PSUM") as pwp2:

                warm_ps = pwp2.tile([128, 512], F32, tag="warm", name="warm")

                def warm(n):
                    for _ in range(n):
                        nc.tensor.matmul(warm_ps[:], ident[:], wrm[:],
                                         start=True, stop=True)

                def out_proj(p, rc, oc):
                    ps = psop.tile([128, 512], F32, tag="psO", name="psO")
                    for mmv in range(8):
                        nc.tensor.matmul(
                            ps[:],
                            ctxP[p][:, rc, mmv, :],
                            wo[:, mmv, 512 * oc:512 * (oc + 1)],
                            start=(mmv == 0), stop=(mmv == 7))
                    outsb = osp.tile([128, 512], F32, tag="outsb",
                                     name="outsb")
                    nc.vector.tensor_tensor(
                        outsb[:], ps[:],
                        bo_bc[:, 512 * oc:512 * (oc + 1)], ALU.add)
                    nc.sync.dma_start(
                        out_d[RP * p + 128 * rc:RP * p + 128 * (rc + 1),
                              512 * oc:512 * (oc + 1)],
                        outsb[:])

                # deferred emissions: [countdown_in_t2_steps, fn]
                pending = []

                def drain():
                    for item in pending[:]:
                        item[0] -= 1
                        if item[0] <= 0:
                            pending.remove(item)
                            item[1]()

                # V projection p=1 + scramble, spread over the early
                # attention stages (psO bank is idle there; out-proj
                # deferrals only begin after the first rc completes)
                for i, (oc, rc) in enumerate([(0, 2), (0, 3), (1, 2), (1, 3)]):
                    pending.append(
                        [1 + 3 * i,
                         lambda oc=oc, rc=rc: v_group(oc, rc, psop, "psO")])
                pending.append([13, lambda: v_scramble(1)])

                def mk_divide(p, rc, box):
                    def divide(jh2):
                        recS = box[0]
                        # rec rows for both g at partitions 0/32,
                        # then ONE PE matmul with the gsel selector
                        # broadcasts them to partitions 0-63 /
                        # 64-127 in PSUM
                        reck2 = reck2s[jh2]
                        for g in range(2):
                            k4 = 2 * jh2 + g
                            nc.vector.tensor_copy(
                                reck2[32 * g:32 * g + 1, :],
                                recS[32 * k4:32 * k4 + 1, :])
                        rbc_ps = psop.tile([128, 512], F32,
                                           tag="psO", name="rbcps")
                        nc.tensor.matmul(
                            rbc_ps[:], gsel[:], reck2[0:33, :],
                            start=True, stop=True)
                        for g in range(2):
                            dst = ctxP[p][64 * g:64 * (g + 1), rc, :,
                                          64 * jh2:64 * (jh2 + 1)]
                            nc.vector.tensor_tensor(
                                dst.rearrange(
                                    "c m (b rr) -> c m b rr", b=4),
                                dst.rearrange(
                                    "c m (b rr) -> c m b rr", b=4),
                                rbc_ps[64 * g:64 * (g + 1), :]
                                .rearrange(
                                    "c (b m rr) -> c m b rr",
                                    b=4, m=8),
                                ALU.mult)
                    return divide

                pending_tail = []
                for p in range(2):
                    denS = None
                    # descending j5: pairs (3,2) then (1,0), so each
                    # pair-completion divide is covered by a LONG next
                    # group (the p-transition lands on nt2=8, not 2)
                    for j5 in (3, 2, 1, 0):
                        nt2 = 2 * (j5 + 1)   # pairs of 128-wide k blocks
                        jh = j5 % 2
                        if jh == 1:
                            # 4 denominator rows (jh, g) staged on
                            # separate partitions; the jh=1 half's
                            # reciprocal + divide fire as soon as THIS
                            # group's tail lands (during the next group),
                            # pulling two hops out of the final chain
                            denS = mp.tile([128, 512], F32, tag="denS",
                                           name="denS")
                        ctx_ps = [pcp.tile([65, 512], F32, tag="ctxps",
                                           name="ctxps")
                                  for _ in range(2)]
                        pts = [None] * nt2

                        def s_stage_g(t2, g):
                            # S for one g: two 512-col halves into one
                            # [128,1024] PSUM tile; diagonal k-blocks only
                            # stream the causal q range (cols >= the
                            # block's position offset) and get the shared
                            # [128,128] triangle mask via an N=128
                            # identity matmul, emitted after both S halves
                            # so the same-bank accumulation never waits on
                            # an undrained S write.
                            st = pssp.tile([128, 1024], F32, tag="st",
                                           name="st")
                            pt = ptp.tile([128, 1024], DT_MM, tag="pt",
                                          name="pt")
                            diag = t2 >= 2 * j5
                            for half in range(2):
                                kb = 2 * t2 + half
                                off = 128 * (kb - 4 * j5) if diag else 0
                                nc.tensor.matmul(
                                    st[:, 512 * half + off:
                                       512 * (half + 1)],
                                    ksc[64 * g:64 * (g + 1),
                                        2048 * p + 128 * kb:
                                        2048 * p + 128 * (kb + 1)],
                                    qsc[64 * g:64 * (g + 1),
                                        2048 * p + 512 * j5 + off:
                                        2048 * p + 512 * (j5 + 1)],
                                    start=True, stop=not diag)
                            if diag:
                                for half in range(2):
                                    kb = 2 * t2 + half
                                    off = 128 * (kb - 4 * j5)
                                    nc.tensor.matmul(
                                        st[:, 512 * half + off:
                                           512 * half + off + 128],
                                        ident[:], masks[:],
                                        start=False, stop=True)
                                for half in range(2):
                                    off = 128 * (2 * t2 + half - 4 * j5)
                                    nc.scalar.activation(
                                        pt[:, 512 * half + off:
                                           512 * (half + 1)],
                                        st[:, 512 * half + off:
                                           512 * (half + 1)], ACTF.Exp)
                            else:
                                nc.scalar.activation(pt[:], st[:], ACTF.Exp)
                            return pt

                        def ctx_stage_g(t2, g, pts=pts, ctx_ps=ctx_ps, p=p,
                                        nt2=nt2, j5=j5):
                            for half in range(2):
                                kb = 2 * t2 + half
                                off = (128 * (kb - 4 * j5)
                                       if kb >= 4 * j5 else 0)
                                nc.tensor.matmul(
                                    ctx_ps[g][:, off:512],
                                    vsc[p][g][:, kb, :],
                                    pts[t2][g][:, 512 * half + off:
                                               512 * (half + 1)],
                                    start=(kb == 0),
                                    stop=(kb == 2 * nt2 - 1))

                        # one-stage software pipeline, interleaved per g:
                        # [S_g0(t2), ctx_g0(t2-1), S_g1(t2), ctx_g1(t2-1)].
                        # S_g(t2)'s PSUM bank frees when exp(t2-1, g)
                        # completes, so the g0 work is compute-ready
                        # mid-stage and exp(t2, g0) starts the moment the
                        # ACT queue frees - ACT never idles.
                        for t2 in range(nt2):
                            pts[t2] = [None, None]
                            for g in range(2):
                                pts[t2][g] = s_stage_g(t2, g)
                                if t2 == 0 and g == 0 and pending_tail:
                                    # previous group's last ctx + evacuation
                                    # runs under this group's first S tiles
                                    pending_tail.pop()()
                                if t2 >= 1:
                                    ctx_stage_g(t2 - 1, g)
                            # one filler matmul per stage: the ACT-bound
                            # cadence leaves the PE under the HAM activity
                            # threshold on the lighter (diagonal/short)
                            # stages, and a single re-throttle costs far
                            # more than 215ns/stage of filler; the short
                            # (j5<=1) groups are lighter (diag-partial
                            # streams + deferred work) and need a second
                            # filler to stay above the HAM threshold
                            warm(2 if j5 <= 1 else 1)
                            drain()

                        def group_tail(j5=j5, jh=jh, ctx_ps=ctx_ps,
                                       nt2=nt2, denS=denS, p=p,
                                       ctx_stage_g=ctx_stage_g):
                            for g in range(2):
                                ctx_stage_g(nt2 - 1, g)
                            # evacuate PSUM fast (frees the ctx banks for
                            # the next group); the reciprocal/divide runs
                            # later, overlapped under later compute
                            for g in range(2):
                                # [c, rc, m, 64jh + 16b + rrlo] <- ctx col
                                # (128b + 16m + rrlo)
                                nc.vector.tensor_copy(
                                    ctxP[p][64 * g:64 * (g + 1), j5 // 2, :,
                                            64 * jh:64 * (jh + 1)]
                                    .rearrange("c m (b rr) -> c m b rr",
                                               b=4),
                                    ctx_ps[g][0:64, :].rearrange(
                                        "c (b m rr) -> c m b rr",
                                        b=4, m=8))
                                nc.vector.tensor_copy(
                                    denS[32 * (2 * jh + g):
                                         32 * (2 * jh + g) + 1, :],
                                    ctx_ps[g][64:65, :])

                        pending_tail.append(group_tail)

                        if jh == 0:
                            rc = j5 // 2
                            box = []

                            def mkrec(denS=denS, box=box):
                                recS = mp.tile([128, 512], F32, tag="recS",
                                               name="recS")
                                nc.vector.reciprocal_approx_fast(
                                    recS[:], denS[:])
                                box.append(recS)

                            div = mk_divide(p, rc, box)
                            if p == 0:
                                cds = (3, 3, 5, 7, 9)
                            else:
                                cds = (2, 2, 3, 4, 6)
                            pending.append([cds[0], mkrec])
                            pending.append([cds[1], lambda d=div: d(0)])
                            pending.append([cds[2], lambda d=div: d(1)])
                            pending.append(
                                [cds[3],
                                 lambda p=p, rc=rc: out_proj(p, rc, 0)])
                            pending.append(
                                [cds[4],
                                 lambda p=p, rc=rc: out_proj(p, rc, 1)])
                while pending_tail:
                    warm(4)
                    pending_tail.pop()()
                for item in pending:
                    warm(6)
                    item[1]()
                warm(4)

    nc.compile()
    return nc


def _get_nc():
    key = "nc"
    if key not in _cache:
        _cache[key] = _build()
    return _cache[key]


def pack_in_maps(x, Wq, bq, Wk, bk, Wv, bv, Wo, bo):
    BF = ml_dtypes.bfloat16
    x = np.asarray(x, np.float32)
    WqT = np.ascontiguousarray(np.asarray(Wq, np.float32).T.astype(BF))
    WkT = np.ascontiguousarray(np.asarray(Wk, np.float32).T.astype(BF))
    WvT = np.ascontiguousarray(np.asarray(Wv, np.float32).T.astype(BF))
    # woTre[64g + d, m, o] = Wo[o, 512g + 64m + d]
    WoTre = np.ascontiguousarray(
        np.asarray(Wo, np.float32).T.reshape(2, 8, 64, E).transpose(0, 2, 1, 3)
        .reshape(128, 8, E).astype(BF))
    bqT = np.ascontiguousarray(np.asarray(bq, np.float32).reshape(8, 128).T)
    bkT8 = np.ascontiguousarray((np.asarray(bk, np.float32) / 8.0).reshape(8, 128).T)
    bvrow = np.asarray(bv, np.float32).reshape(1, E)
    borow = np.asarray(bo, np.float32).reshape(1, E)
    # position-sorted 128-blocks, m-major in-block: index i = 16m + rr%16
    # has in-block position 8*(i%16) + i//16; one mask covers every
    # diagonal block
    ii = np.arange(128)[:, None]
    cc = np.arange(128)[None, :]
    pos_k = 8 * (ii % 16) + ii // 16
    pos_q = 8 * (cc % 16) + cc // 16
    masks = np.where(pos_k <= pos_q, 0.0, NEG).astype(BF)
    ident = np.eye(128).astype(BF)
    gsel = np.zeros((33, 128), np.float32)
    gsel[0, 0:64] = 1.0
    gsel[32, 64:128] = 1.0
    gsel = gsel.astype(BF)

    in_maps = []
    for c in range(8):
        xTs = np.empty((E, R), BF)
        for p in range(2):
            h = 2 * c + p
            b_, mp_ = divmod(h, 8)
            xTs[:, RP * p:RP * (p + 1)] = x[b_, RP * mp_:RP * (mp_ + 1), :].T.astype(BF)
        in_maps.append({
            "xT": np.ascontiguousarray(xTs), "wqT": WqT, "wkT": WkT,
            "wvT": WvT, "woTre": WoTre, "bqT": bqT, "bkT8": bkT8,
            "bvrow": bvrow, "borow": borow, "masks": masks, "ident": ident,
            "gsel": gsel,
        })
    return in_maps


def unpack_out(results):
    out = np.empty((2, 2048, E), np.float32)
    for c in range(8):
        o = results[c]["out"]
        for p in range(2):
            h = 2 * c + p
            b_, mp_ = divmod(h, 8)
            out[b_, RP * mp_:RP * (mp_ + 1), :] = o[RP * p:RP * (p + 1), :]
    return out


def kernel(x, Wq, bq, Wk, bk, Wv, bv, Wo, bo):
    in_maps = pack_in_maps(x, Wq, bq, Wk, bk, Wv, bv, Wo, bo)
    nc = _get_nc()
    res = run_bass_kernel_spmd(nc, in_maps, core_ids=list(range(8)))
    return unpack_out(res.results)


# revision 59
# speedup vs baseline: 1.2893x; 1.0175x over previous
"""MicroHeadAttention Trainium2 kernel (8-core SPMD, data-parallel over
(batch, row-chunk) pairs).

Shapes (hardcoded): x (2, 2048, 1024), weights (1024, 1024), biases (1024,).
EMBED=1024, 16 heads in 2 blocks (g) of 8 micro-heads, head_dim 64.

Decomposition: the reference's "scramble" is a raw row-major reshape, so the
attention head (b, g, m') consumes exactly rows x[b, 256m':256(m'+1)] and
weight columns [512g:512(g+1)], reshaped (256, 512) -> (2048, 64) with
scrambled position n' = 8*row + m (m = 64-channel sub-block).  16 (b, m')
row-chunks across 8 cores = 2 per core; each chunk has g=0,1 -> 4 heads/core.

Final schedule (~196us max / ~193us mean per-core NEFF, vs 234-249us
baseline).  Layout: position-sorted 128-blocks with m-major order INSIDE
each block (col = 128*(rr//16) + 16*m + rr%16) - block-triangular
causality (diagonal S/exp/ctx skip the above-diagonal quarter, one shared
[128,128] triangle mask) while projection PSUM evacuations still write
16-element-contiguous runs (pure n'-order makes them 3.8x-slower stride-8
scatters).  One filler matmul per attention stage keeps the PE above the
HAM clock-gate activity threshold (a single re-throttle to 1.2GHz costs
20-60us; both fewer and more fillers measured slower).  Schedule notes:
  - ACT (exp) is the bottleneck engine of the attention phase (~91us of
    exp at ~1.1us per [128,1024] tile); everything is arranged so ACT never
    waits: per-g S stages and ctx stages are interleaved
    [S_g0(t2+1), ctx_g0(t2), S_g1(t2+1), ctx_g1(t2)] so the g0 tiles of the
    next stage (whose PSUM banks free when exp(t2, g0) completes mid-stage)
    are compute-ready the moment the ACT queue frees up.
  - exp stays at [128,1024] per-g granularity: splitting it per half costs
    ~155ns/instruction of ACT overhead (+20us measured in v3).
  - all weight DMAs start up front from persistent tiles (single 2MB
    transfers); no pool-reuse dependencies anywhere.
  - the V projection for the second row-pair (p=1) is deferred into the
    early attention phase (PE slack under the ACT-bound cadence), shrinking
    the serial projection prefix.
  - softmax divide: denominator rows batched on partitions 0/32/64/96,
    one custom-DVE reciprocal_approx_fast per rc (5x cheaper than
    InstReciprocal), a gsel selector matmul broadcasts the rec rows on
    the PE, DVE multiplies them into ctxP.
  - deferred out-proj / rbc emissions are staggered so consecutive users of
    the single psO bank never stall the PE FIFO; the final drain interleaves
    keep-warm matmuls on a spare PSUM bank so the latency-bound tail chain
    runs at 2.4GHz.
"""

import ml_dtypes
import numpy as np

import concourse.bass as bass
import concourse.mybir as mybir
from concourse import bacc
from concourse.tile import TileContext
from concourse.bass_utils import run_bass_kernel_spmd

F32 = mybir.dt.float32
BF16 = mybir.dt.bfloat16
DT_MM = BF16
NEG = -1e30
E = 1024
R = 512       # rows per core
RP = 256      # rows per pair
ALU = mybir.AluOpType
ACTF = mybir.ActivationFunctionType

_cache = {}


def _build():
    nc = bacc.Bacc()
    xT_d = nc.dram_tensor("xT", (E, R), DT_MM, kind="ExternalInput")
    wq_d = nc.dram_tensor("wqT", (E, E), DT_MM, kind="ExternalInput")
    wk_d = nc.dram_tensor("wkT", (E, E), DT_MM, kind="ExternalInput")
    wv_d = nc.dram_tensor("wvT", (E, E), DT_MM, kind="ExternalInput")
    wo_d = nc.dram_tensor("woTre", (128, 8, E), DT_MM, kind="ExternalInput")
    bq_d = nc.dram_tensor("bqT", (128, 8), F32, kind="ExternalInput")
    bk_d = nc.dram_tensor("bkT8", (128, 8), F32, kind="ExternalInput")
    bv_d = nc.dram_tensor("bvrow", (1, E), F32, kind="ExternalInput")
    bo_d = nc.dram_tensor("borow", (1, E), F32, kind="ExternalInput")
    masks_d = nc.dram_tensor("masks", (128, 128), DT_MM, kind="ExternalInput")
    ident_d = nc.dram_tensor("ident", (128, 128), DT_MM, kind="ExternalInput")
    gsel_d = nc.dram_tensor("gsel", (33, 128), DT_MM, kind="ExternalInput")
    out_d = nc.dram_tensor("out", (R, E), F32, kind="ExternalOutput")

    with TileContext(nc) as tc:
        with (
            tc.tile_pool(name="persist", bufs=1) as pp,
            tc.tile_pool(name="pt", bufs=4) as ptp,
            tc.tile_pool(name="misc", bufs=2) as mp,
            tc.tile_pool(name="outs", bufs=4) as osp,
            tc.tile_pool(name="dram", bufs=1, space="DRAM") as dp,
        ):
            # ---- persistent tiles ----
            bqT = pp.tile([128, 8], F32, tag="bqT", name="bqT")
            bkT8 = pp.tile([128, 8], F32, tag="bkT8", name="bkT8")
            # n'-contiguous layout: qsc/ksc/vsc columns are sorted by the
            # scrambled position n' = 8*rr + m, so causality is
            # block-triangular: k-blocks strictly below the diagonal are
            # fully visible and the single [128,128] upper-triangle mask
            # covers every diagonal block.
            masks = pp.tile([128, 128], DT_MM, tag="masks", name="masks")
            # dependency-free all-zeros warm operand (memset, no DMA)
            wrm = pp.tile([128, 512], DT_MM, tag="wrm", name="wrm")
            ident = pp.tile([128, 128], DT_MM, tag="ident", name="ident")
            gsel = pp.tile([33, 128], DT_MM, tag="gsel", name="gsel")
            # persistent rec-row staging (rows 1-31 stay zero so the 33-wide
            # gsel broadcast matmul never reads uninitialized SBUF)
            reck2s = [pp.tile([33, 512], DT_MM, tag=f"reck2{i}",
                              name=f"reck2{i}") for i in range(2)]
            qsc = pp.tile([128, 4096], DT_MM, tag="qsc", name="qsc")
            ksc = pp.tile([128, 4096], DT_MM, tag="ksc", name="ksc")
            vsc = [[pp.tile([128, 16, 65], DT_MM, tag=f"vsc{p}{g}", name=f"vsc{p}{g}")
                    for g in range(2)] for p in range(2)]
            # ctxP[p][c, rc, m, rr] : out-proj lhsT slices are contiguous
            # (FWL needs a single-stride stationary AP); with m-major ctx
            # columns the divide writes 64-contiguous runs into it
            ctxP = [pp.tile([128, 2, 8, 128], DT_MM, tag=f"ctxP{p}", name=f"ctxP{p}")
                    for p in range(2)]
            vtmp = dp.tile([2, 2, 2048, 64], DT_MM, tag="vtmp", name="vtmp")

            xt = pp.tile([128, 8, R], DT_MM, tag="xt", name="xt")
            wq = pp.tile([128, 8, E], DT_MM, tag="wq", name="wq")
            wk = pp.tile([128, 8, E], DT_MM, tag="wk", name="wk")
            wv = pp.tile([128, 8, E], DT_MM, tag="wv", name="wv")
            wo = pp.tile([128, 8, E], DT_MM, tag="wo", name="wo")
            vnat = [pp.tile([128, 2, E], DT_MM, tag=f"vnat{p}", name=f"vnat{p}")
                    for p in range(2)]
            bvr = pp.tile([1, E], F32, tag="bvr", name="bvr")
            bv_bc = pp.tile([128, E], F32, tag="bvbc", name="bvbc")
            bor = pp.tile([1, E], F32, tag="bor", name="bor")
            bo_bc = pp.tile([128, E], F32, tag="bobc", name="bobc")

            # masks/ident are host constants so the PE pre-warm only
            # waits on these two small DMAs
            nc.sync.dma_start(masks[:], masks_d[:])
            nc.sync.dma_start(ident[:], ident_d[:])
            nc.sync.dma_start(gsel[:], gsel_d[:])
            nc.sync.dma_start(bqT[:], bq_d[:])
            nc.sync.dma_start(bkT8[:], bk_d[:])
            nc.sync.dma_start(bvr[:], bv_d[:])
            nc.sync.dma_start(bor[:], bo_d[:])

            nc.sync.dma_start(xt[:], xT_d.rearrange("(ko ki) r -> ki ko r", ki=128))
            wv_v = wv_d.rearrange("(ko ki) o -> ki ko o", ki=128)
            # oc-major so V's first output half can start after 1 MB
            for oc in range(2):
                nc.sync.dma_start(wv[:, :, 512 * oc:512 * (oc + 1)],
                                  wv_v[:, :, 512 * oc:512 * (oc + 1)])
            nc.sync.dma_start(wq[:], wq_d.rearrange("(ko ki) o -> ki ko o", ki=128))
            nc.sync.dma_start(wk[:], wk_d.rearrange("(ko ki) o -> ki ko o", ki=128))
            nc.sync.dma_start(wo[:], wo_d[:])

            nc.gpsimd.partition_broadcast(bv_bc[:], bvr[:])
            nc.gpsimd.partition_broadcast(bo_bc[:], bor[:])
            ones16 = pp.tile([128, 16], F32, tag="ones16", name="ones16")
            nc.gpsimd.memset(ones16[:], 1.0)
            nc.vector.memset(wrm[:], 0.0)
            for i in range(2):
                nc.vector.memset(reck2s[i][:], 0.0)
            for p in range(2):
                for g in range(2):
                    nc.vector.tensor_copy(vsc[p][g][:, :, 64], ones16[:])

            def v_group(oc, rc, psp_, tag="psA"):
                p, half = rc // 2, rc % 2
                ps = psp_.tile([128, 512], F32, tag=tag, name="psA")
                for ki in range(8):
                    nc.tensor.matmul(
                        ps[:], xt[:, ki, 128 * rc:128 * (rc + 1)],
                        wv[:, ki, 512 * oc:512 * (oc + 1)],
                        start=(ki == 0), stop=(ki == 7))
                nc.vector.tensor_tensor(
                    vnat[p][:, half, 512 * oc:512 * (oc + 1)],
                    ps[:], bv_bc[:, 512 * oc:512 * (oc + 1)], ALU.add)

            def v_scramble(p):
                for g in range(2):
                    # n' = 8*(128h + 64rb + rr) + m = 1024h + 512rb
                    # + 8rr + m.  Split per (h, rb): the DMA balancer
                    # tops out at 3 dims.
                    dstv = vtmp[p, g].rearrange(
                        "(h rb rr m) d -> h rb rr m d", h=2, rb=2, m=8)
                    for h in range(2):
                        for rb in range(2):
                            srcs = vnat[p][64 * rb:64 * (rb + 1), h,
                                           512 * g:512 * (g + 1)]
                            nc.sync.dma_start(
                                dstv[h, rb],
                                srcs.rearrange("rr (m d) -> rr m d", m=8))
                for g in range(2):
                    # vsc partition index is the in-block m-major coord
                    # 16m + rr%16; vtmp rows are flat n' = 128 kb + 8 rr
                    # + m, so gather per m to keep the AP affine
                    src_v = vtmp[p, g].rearrange(
                        "(kb rr mm) d -> rr mm kb d", kb=16, rr=16)
                    for m in range(8):
                        nc.sync.dma_start(
                            vsc[p][g][16 * m:16 * (m + 1), :, 0:64],
                            src_v[:, m])

            with tc.tile_pool(name="ps1", bufs=5, space="PSUM") as psp, \
                 tc.tile_pool(name="pswarm", bufs=1, space="PSUM") as pwp:
                # ---- PE pre-warm: back-to-back matmuls on the mask tile
                # keep the HAM activity window busy while the input DMAs
                # stream, so the real matmuls start at 2.4 GHz.
                psw = pwp.tile([128, 512], F32, tag="psw", name="psw")
                for _ in range(24):
                    nc.tensor.matmul(psw[:], ident[:], wrm[:],
                                     start=True, stop=True)

                def qk_proj(w_tile, bias_tile, scale, dst):
                    for t in range(8):
                        ps = psp.tile([128, 512], F32, tag="psA", name="psA")
                        for ki in range(8):
                            nc.tensor.matmul(
                                ps[:], w_tile[:, ki, 128 * t:128 * (t + 1)],
                                xt[:, ki, :], start=(ki == 0), stop=(ki == 7))
                        g, u = t // 4, t % 4
                        for mh in range(2):
                            mmv = 2 * u + mh
                            # position-sorted 128-blocks (b = rr//16),
                            # m-major inside the block: col = 2048 pp
                            # + 128 b + 16 m + rr%16 - head m's channels
                            # land in contiguous 16-element runs
                            dest = dst.rearrange(
                                "c (pp b mm rrlo) -> c pp b mm rrlo",
                                pp=2, b=16, mm=8, rrlo=16)[
                                64 * g:64 * (g + 1), :, :, mmv, :]
                            src = ps[64 * mh:64 * (mh + 1), :].rearrange(
                                "c (pp b rrlo) -> c pp b rrlo",
                                pp=2, b=16)
                            # VectorE lanes are partition-locked: it can only
                            # take the copies whose src/dst partition ranges
                            # line up (g == mh); ScalarE handles the crossed
                            # ones.
                            if mh != g:
                                nc.scalar.activation(
                                    dest, src, ACTF.Identity,
                                    bias=bias_tile[64 * mh:64 * (mh + 1), t:t + 1],
                                    scale=scale)
                            else:
                                nc.vector.tensor_scalar(
                                    out=dest, in0=src, scalar1=scale,
                                    scalar2=bias_tile[64 * mh:64 * (mh + 1), t:t + 1],
                                    op0=ALU.mult, op1=ALU.add)

                # V projection for the first row-pair only; p=1 is deferred
                # into the attention phase (PE slack under the ACT-bound
                # cadence).
                for oc in range(2):
                    for rc in range(2):
                        v_group(oc, rc, psp)
                v_scramble(0)

                qk_proj(wq, bqT, 1.0, qsc)
                qk_proj(wk, bkT8, 0.125, ksc)

            # ---- attention + interleaved output projection ----
            with tc.tile_pool(name="psS", bufs=2, space="PSUM") as pssp, \
                 tc.tile_pool(name="psctx", bufs=2, space="PSUM") as pcp, \
                 tc.tile_pool(name="psO", bufs=1, space="PSUM") as psop, \
                 tc.tile_pool(name="pswarm2", bufs=1, space="PSUM") as pwp2:

                warm_ps = pwp2.tile([128, 512], F32, tag="warm", name="warm")

                def warm(n):
                    for _ in range(n):
                        nc.tensor.matmul(warm_ps[:], ident[:], wrm[:],
                                         start=True, stop=True)

                def out_proj(p, rc, oc, split=False):
                    ps = psop.tile([128, 512], F32, tag="psO", name="psO")
                    if split:
                        # flush-chain variant: output rows [64*jh2,
                        # 64*(jh2+1)) depend only on divide(jh2), so the
                        # jh2=0 half's matmuls (col-tile (0,0)) run while
                        # divide(1)'s DVE multiplies are still in flight
                        for jh2 in range(2):
                            for mmv in range(8):
                                nc.tensor.matmul(
                                    ps[64 * jh2:64 * (jh2 + 1), :],
                                    ctxP[p][:, rc, mmv,
                                            64 * jh2:64 * (jh2 + 1)],
                                    wo[:, mmv, 512 * oc:512 * (oc + 1)],
                                    start=(mmv == 0), stop=(mmv == 7))
                    else:
                        for mmv in range(8):
                            nc.tensor.matmul(
                                ps[:],
                                ctxP[p][:, rc, mmv, :],
                                wo[:, mmv, 512 * oc:512 * (oc + 1)],
                                start=(mmv == 0), stop=(mmv == 7))
                    outsb = osp.tile([128, 512], F32, tag="outsb",
                                     name="outsb")
                    nc.vector.tensor_tensor(
                        outsb[:], ps[:],
                        bo_bc[:, 512 * oc:512 * (oc + 1)], ALU.add)
                    nc.sync.dma_start(
                        out_d[RP * p + 128 * rc:RP * p + 128 * (rc + 1),
                              512 * oc:512 * (oc + 1)],
                        outsb[:])

                # deferred emissions: [countdown_in_t2_steps, fn]
                pending = []

                def drain():
                    for item in pending[:]:
                        item[0] -= 1
                        if item[0] <= 0:
                            pending.remove(item)
                            item[1]()

                # V projection p=1 + scramble, spread over the early
                # attention stages (psO bank is idle there; out-proj
                # deferrals only begin after the first rc completes)
                for i, (oc, rc) in enumerate([(0, 2), (0, 3), (1, 2), (1, 3)]):
                    pending.append(
                        [1 + 3 * i,
                         lambda oc=oc, rc=rc: v_group(oc, rc, psop, "psO")])
                pending.append([13, lambda: v_scramble(1)])

                def mk_divide(p, rc, box):
                    def divide(jh2):
                        recS = box[0]
                        # rec rows for both g at partitions 0/32,
                        # then ONE PE matmul with the gsel selector
                        # broadcasts them to partitions 0-63 /
                        # 64-127 in PSUM
                        reck2 = reck2s[jh2]
                        for g in range(2):
                            k4 = 2 * jh2 + g
                            nc.vector.tensor_copy(
                                reck2[32 * g:32 * g + 1, :],
                                recS[32 * k4:32 * k4 + 1, :])
                        rbc_ps = psop.tile([128, 512], F32,
                                           tag="psO", name="rbcps")
                        nc.tensor.matmul(
                            rbc_ps[:], gsel[:], reck2[0:33, :],
                            start=True, stop=True)
                        for g in range(2):
                            dst = ctxP[p][64 * g:64 * (g + 1), rc, :,
                                          64 * jh2:64 * (jh2 + 1)]
                            nc.vector.tensor_tensor(
                                dst.rearrange(
                                    "c m (b rr) -> c m b rr", b=4),
                                dst.rearrange(
                                    "c m (b rr) -> c m b rr", b=4),
                                rbc_ps[64 * g:64 * (g + 1), :]
                                .rearrange(
                                    "c (b m rr) -> c m b rr",
                                    b=4, m=8),
                                ALU.mult)
                    return divide

                pending_tail = []
                for p in range(2):
                    denS = None
                    # descending j5: pairs (3,2) then (1,0), so each
                    # pair-completion divide is covered by a LONG next
                    # group (the p-transition lands on nt2=8, not 2)
                    for j5 in (3, 2, 1, 0):
                        nt2 = 2 * (j5 + 1)   # pairs of 128-wide k blocks
                        jh = j5 % 2
                        if jh == 1:
                            # 4 denominator rows (jh, g) staged on
                            # separate partitions; the jh=1 half's
                            # reciprocal + divide fire as soon as THIS
                            # group's tail lands (during the next group),
                            # pulling two hops out of the final chain
                            denS = mp.tile([128, 512], F32, tag="denS",
                                           name="denS")
                        ctx_ps = [pcp.tile([65, 512], F32, tag="ctxps",
                                           name="ctxps")
                                  for _ in range(2)]
                        pts = [None] * nt2

                        def s_stage_g(t2, g):
                            # S for one g: two 512-col halves into one
                            # [128,1024] PSUM tile; diagonal k-blocks only
                            # stream the causal q range (cols >= the
                            # block's position offset) and get the shared
                            # [128,128] triangle mask via an N=128
                            # identity matmul, emitted after both S halves
                            # so the same-bank accumulation never waits on
                            # an undrained S write.
                            st = pssp.tile([128, 1024], F32, tag="st",
                                           name="st")
                            pt = ptp.tile([128, 1024], DT_MM, tag="pt",
                                          name="pt")
                            diag = t2 >= 2 * j5
                            for half in range(2):
                                kb = 2 * t2 + half
                                off = 128 * (kb - 4 * j5) if diag else 0
                                nc.tensor.matmul(
                                    st[:, 512 * half + off:
                                       512 * (half + 1)],
                                    ksc[64 * g:64 * (g + 1),
                                        2048 * p + 128 * kb:
                                        2048 * p + 128 * (kb + 1)],
                                    qsc[64 * g:64 * (g + 1),
                                        2048 * p + 512 * j5 + off:
                                        2048 * p + 512 * (j5 + 1)],
                                    start=True, stop=not diag)
                            if diag:
                                for half in range(2):
                                    kb = 2 * t2 + half
                                    off = 128 * (kb - 4 * j5)
                                    nc.tensor.matmul(
                                        st[:, 512 * half + off:
                                           512 * half + off + 128],
                                        ident[:], masks[:],
                                        start=False, stop=True)
                                for half in range(2):
                                    off = 128 * (2 * t2 + half - 4 * j5)
                                    nc.scalar.activation(
                                        pt[:, 512 * half + off:
                                           512 * (half + 1)],
                                        st[:, 512 * half + off:
                                           512 * (half + 1)], ACTF.Exp)
                            else:
                                nc.scalar.activation(pt[:], st[:], ACTF.Exp)
                            return pt

                        def ctx_stage_g(t2, g, pts=pts, ctx_ps=ctx_ps, p=p,
                                        nt2=nt2, j5=j5):
                            for half in range(2):
                                kb = 2 * t2 + half
                                off = (128 * (kb - 4 * j5)
                                       if kb >= 4 * j5 else 0)
                                nc.tensor.matmul(
                                    ctx_ps[g][:, off:512],
                                    vsc[p][g][:, kb, :],
                                    pts[t2][g][:, 512 * half + off:
                                               512 * (half + 1)],
                                    start=(kb == 0),
                                    stop=(kb == 2 * nt2 - 1))

                        # one-stage software pipeline, interleaved per g:
                        # [S_g0(t2), ctx_g0(t2-1), S_g1(t2), ctx_g1(t2-1)].
                        # S_g(t2)'s PSUM bank frees when exp(t2-1, g)
                        # completes, so the g0 work is compute-ready
                        # mid-stage and exp(t2, g0) starts the moment the
                        # ACT queue frees - ACT never idles.
                        for t2 in range(nt2):
                            pts[t2] = [None, None]
                            for g in range(2):
                                pts[t2][g] = s_stage_g(t2, g)
                                if t2 == 0 and g == 0 and pending_tail:
                                    # previous group's last ctx + evacuation
                                    # runs under this group's first S tiles
                                    pending_tail.pop()()
                                if t2 >= 1:
                                    ctx_stage_g(t2 - 1, g)
                            # one filler matmul per stage: the ACT-bound
                            # cadence leaves the PE under the HAM activity
                            # threshold on the lighter (diagonal/short)
                            # stages, and a single re-throttle costs far
                            # more than 215ns/stage of filler
                            warm(1)
                            drain()

                        def group_tail(j5=j5, jh=jh, ctx_ps=ctx_ps,
                                       nt2=nt2, denS=denS, p=p,
                                       ctx_stage_g=ctx_stage_g):
                            for g in range(2):
                                ctx_stage_g(nt2 - 1, g)
                            # evacuate PSUM fast (frees the ctx banks for
                            # the next group); the reciprocal/divide runs
                            # later, overlapped under later compute
                            for g in range(2):
                                # [c, rc, m, 64jh + 16b + rrlo] <- ctx col
                                # (128b + 16m + rrlo)
                                nc.vector.tensor_copy(
                                    ctxP[p][64 * g:64 * (g + 1), j5 // 2, :,
                                            64 * jh:64 * (jh + 1)]
                                    .rearrange("c m (b rr) -> c m b rr",
                                               b=4),
                                    ctx_ps[g][0:64, :].rearrange(
                                        "c (b m rr) -> c m b rr",
                                        b=4, m=8))
                                nc.vector.tensor_copy(
                                    denS[32 * (2 * jh + g):
                                         32 * (2 * jh + g) + 1, :],
                                    ctx_ps[g][64:65, :])

                        pending_tail.append(group_tail)

                        if jh == 0:
                            rc = j5 // 2
                            box = []

                            def mkrec(denS=denS, box=box):
                                recS = mp.tile([128, 512], F32, tag="recS",
                                               name="recS")
                                nc.vector.reciprocal_approx_fast(
                                    recS[:], denS[:])
                                box.append(recS)

                            div = mk_divide(p, rc, box)
                            if p == 0:
                                cds = (3, 3, 5, 7, 9)
                            else:
                                cds = (2, 2, 3, 4, 6)
                            pending.append([cds[0], mkrec])
                            pending.append([cds[1], lambda d=div: d(0)])
                            pending.append([cds[2], lambda d=div: d(1)])
                            pending.append(
                                [cds[3],
                                 lambda p=p, rc=rc: out_proj(
                                     p, rc, 0, split=(p == 1 and rc == 0))])
                            pending.append(
                                [cds[4],
                                 lambda p=p, rc=rc: out_proj(
                                     p, rc, 1, split=(p == 1 and rc == 0))])
                while pending_tail:
                    warm(4)
                    pending_tail.pop()()
                for item in pending:
                    warm(6)
                    item[1]()
                warm(4)

    nc.compile()
    return nc


def _get_nc():
    key = "nc"
    if key not in _cache:
        _cache[key] = _build()
    return _cache[key]


def pack_in_maps(x, Wq, bq, Wk, bk, Wv, bv, Wo, bo):
    BF = ml_dtypes.bfloat16
    x = np.asarray(x, np.float32)
    WqT = np.ascontiguousarray(np.asarray(Wq, np.float32).T.astype(BF))
    WkT = np.ascontiguousarray(np.asarray(Wk, np.float32).T.astype(BF))
    WvT = np.ascontiguousarray(np.asarray(Wv, np.float32).T.astype(BF))
    # woTre[64g + d, m, o] = Wo[o, 512g + 64m + d]
    WoTre = np.ascontiguousarray(
        np.asarray(Wo, np.float32).T.reshape(2, 8, 64, E).transpose(0, 2, 1, 3)
        .reshape(128, 8, E).astype(BF))
    bqT = np.ascontiguousarray(np.asarray(bq, np.float32).reshape(8, 128).T)
    bkT8 = np.ascontiguousarray((np.asarray(bk, np.float32) / 8.0).reshape(8, 128).T)
    bvrow = np.asarray(bv, np.float32).reshape(1, E)
    borow = np.asarray(bo, np.float32).reshape(1, E)
    # position-sorted 128-blocks, m-major in-block: index i = 16m + rr%16
    # has in-block position 8*(i%16) + i//16; one mask covers every
    # diagonal block
    ii = np.arange(128)[:, None]
    cc = np.arange(128)[None, :]
    pos_k = 8 * (ii % 16) + ii // 16
    pos_q = 8 * (cc % 16) + cc // 16
    masks = np.where(pos_k <= pos_q, 0.0, NEG).astype(BF)
    ident = np.eye(128).astype(BF)
    gsel = np.zeros((33, 128), np.float32)
    gsel[0, 0:64] = 1.0
    gsel[32, 64:128] = 1.0
    gsel = gsel.astype(BF)

    in_maps = []
    for c in range(8):
        xTs = np.empty((E, R), BF)
        for p in range(2):
            h = 2 * c + p
            b_, mp_ = divmod(h, 8)
            xTs[:, RP * p:RP * (p + 1)] = x[b_, RP * mp_:RP * (mp_ + 1), :].T.astype(BF)
        in_maps.append({
            "xT": np.ascontiguousarray(xTs), "wqT": WqT, "wkT": WkT,
            "wvT": WvT, "woTre": WoTre, "bqT": bqT, "bkT8": bkT8,
            "bvrow": bvrow, "borow": borow, "masks": masks, "ident": ident,
            "gsel": gsel,
        })
    return in_maps


def unpack_out(results):
    out = np.empty((2, 2048, E), np.float32)
    for c in range(8):
        o = results[c]["out"]
        for p in range(2):
            h = 2 * c + p
            b_, mp_ = divmod(h, 8)
            out[b_, RP * mp_:RP * (mp_ + 1), :] = o[RP * p:RP * (p + 1), :]
    return out


def kernel(x, Wq, bq, Wk, bk, Wv, bv, Wo, bo):
    in_maps = pack_in_maps(x, Wq, bq, Wk, bk, Wv, bv, Wo, bo)
    nc = _get_nc()
    res = run_bass_kernel_spmd(nc, in_maps, core_ids=list(range(8)))
    return unpack_out(res.results)
